# revision 8
# baseline (speedup 1.0000x reference)
"""Trainium2 Bass kernel for nn_ButterflyLayer2D (butterfly 2D CNN).

Strategy: pure data parallel over 8 NeuronCores (16 batch each), with the
per-core batch processed in 2 phases of 8 to fit SBUF.

All tensors are pre-arranged on the host (numpy) into DMA-friendly layouts:
  - activations live in SBUF as [128 = (w%2)*64 + c, (node, b, h, w//2)]
    so each 2x2-stride-2 per-node conv becomes 4 fp32r matmuls with K=128
    chunks: col-group q = output w-parity (tile_position (0, 64q)), x-chunks
    accumulate in PSUM. PSUM [128=(q,c_out), N] is evicted full-width by a
    single relu+bias op (alternating ScalarE/VectorE) directly into the next
    level's interleaved layout — zero data reshuffling anywhere on chip.
  - the input 4x4-patch conv uses the same trick with K=16 row-groups spread
    over 4 partition groups (one per b%4) for PE concurrency.
  - the final per-node dense is a [64,128] x [64,b] matmul; outputs are
    written as [128=(r,ou,ov), (ph,node,b)] and decoded on the host.
Weights are streamed from HBM in 8-node chunks through a recycled tile tag.
"""

import numpy as np
from contextlib import ExitStack

import concourse.bass as bass
import concourse.tile as tile
from concourse import bacc, mybir
from concourse.bass_utils import run_bass_kernel_spmd

F32 = mybir.dt.float32
F32R = mybir.dt.float32r
BF16 = mybir.dt.bfloat16
AF = mybir.ActivationFunctionType
ALU = mybir.AluOpType

B, IN, NLVL, KLVL, C = 128, 256, 6, 3, 64
NK, OU, OV = 8, 8, 8
NCORES = 8
BC = B // NCORES          # 16 per-core batch
PH = 2                    # phases per core
BG = BC // PH             # 8 batch per phase
LVL_NODES = [4, 16, 64, 64, 64, 64]          # nodes per level
LVL_HIN = [64, 32, 16, 8, 4, 2]              # spatial H into each level
WGRP = 8                  # weight streaming chunk (nodes)


# ----------------------------------------------------------------------------
# host-side pre-arrangement
# ----------------------------------------------------------------------------

def _prep_weights(inputs):
    """Weights/biases blobs shared by all cores."""
    out = {}
    # input filter: lhsT [16=(p,q), 64], replicated at partition bases 0/32/64/96
    import ml_dtypes
    fin = inputs["in_filter"][:, :, 0, :].reshape(16, C).astype(np.float32)
    finr = np.zeros((128, C), np.float32)
    for g in range(4):
        finr[g * 32 : g * 32 + 16] = fin
    out["fin"] = finr.astype(ml_dtypes.bfloat16)
    out["bin"] = np.concatenate([inputs["in_bias"], inputs["in_bias"]]).reshape(
        128, 1
    ).astype(np.float32)

    for lvl in range(1, NLVL + 1):
        f = inputs[f"f{lvl}"].astype(np.float32)  # [n,n,2,2,C,C] (x,y,ci,co)
        n = f.shape[0]
        assert n == 2 ** min(lvl, KLVL)
        # lhsT per node: [(y*64+ci), (x*64+co)]
        w = f.transpose(0, 1, 3, 4, 2, 5).reshape(n * n, 2 * C, 2 * C)
        # blob [128, nodes*128], free = (node, x*64+co)
        out[f"w{lvl}"] = np.ascontiguousarray(w.transpose(1, 0, 2)).reshape(
            128, n * n * 128
        ).astype(ml_dtypes.bfloat16)
        b = inputs[f"b{lvl}"].astype(np.float32).reshape(n * n, C)
        if lvl < NLVL:
            # [128, nodes]: rows (q,c) with bias duplicated across q
            bb = np.concatenate([b, b], axis=1)  # [nodes, 128]
            out[f"b{lvl}"] = np.ascontiguousarray(bb.T)
        else:
            # lvl6 node-pair scheme: psum rows = (cA, cB) for pair (2k, 2k+1)
            bb = b.reshape(n * n // 2, 2 * C)  # [pairs, (cA,cB)]
            out[f"b{lvl}"] = np.ascontiguousarray(bb.T)  # [128, 32]
    # dense: lhsT per node [64=c, 128=(r, ou*8+ov)]
    wd = inputs["Wd"].astype(np.float32).reshape(NK * NK, 2, C, OU * OV)
    wd = wd.transpose(2, 0, 1, 3).reshape(C, NK * NK * 2 * OU * OV)
    out["wd"] = np.ascontiguousarray(wd).astype(ml_dtypes.bfloat16)
    return out


def _prep_input(in_data_core):
    """Per-core input blob: [64 = (b%4)*16 + (i%4)*4 + (j%4),
    (ph, b//4%2, x=i//4, y4=j//4)] packed (no zero rows)."""
    ind = in_data_core[:, :, :, 0]  # [16, 256, 256]
    a = ind.reshape(PH, 2, 4, 64, 4, 64, 4)  # [ph, half, g, x, p, y4, q]
    a = a.transpose(2, 4, 6, 0, 1, 3, 5)     # [g, p, q, ph, half, x, y4]
    import ml_dtypes
    return np.ascontiguousarray(a).reshape(64, PH * 2 * 64 * 64).astype(ml_dtypes.bfloat16)


def _decode_output(t2_core):
    """t2 [128=(r,ou,ov), (ph, node, bl)] -> [16, 64, 64, 2]."""
    t = t2_core.reshape(2, OU, OV, PH, NK, NK, BG)  # r,ou,ov,ph,u,v,bl
    t = t.transpose(3, 6, 4, 1, 5, 2, 0)            # ph,bl,u,ou,v,ov,r
    return np.ascontiguousarray(t).reshape(BC, NK * OU, NK * OV, 2)


# ----------------------------------------------------------------------------
# device kernel
# ----------------------------------------------------------------------------

def _build_kernel():
    nc = bacc.Bacc(None, target_bir_lowering=False)
    p = {}
    p["a0"] = nc.declare_dram_parameter("a0", [64, PH * 2 * 64 * 64], BF16, isOutput=False)
    p["fin"] = nc.declare_dram_parameter("fin", [128, C], BF16, isOutput=False)
    p["bin"] = nc.declare_dram_parameter("bin", [128, 1], F32, isOutput=False)
    for lvl in range(1, NLVL + 1):
        n2 = LVL_NODES[lvl - 1]
        p[f"w{lvl}"] = nc.declare_dram_parameter(f"w{lvl}", [128, n2 * 128], BF16, isOutput=False)
        bcols = n2 if lvl < NLVL else n2 // 2
        p[f"b{lvl}"] = nc.declare_dram_parameter(f"b{lvl}", [128, bcols], F32, isOutput=False)
    p["wd"] = nc.declare_dram_parameter("wd", [64, NK * NK * 128], BF16, isOutput=False)
    t2 = nc.declare_dram_parameter("t2", [128, PH * NK * NK * BG], F32, isOutput=True)

    evict_ctr = [0]

    def evict(out_ap, psum_ap, bias_ap):
        """relu(psum + bias) -> sbuf, alternating engines to split the load."""
        evict_ctr[0] += 1
        if evict_ctr[0] % 2 == 0:
            nc.scalar.activation(out_ap, psum_ap, AF.Relu, bias=bias_ap)
        else:
            nc.vector.tensor_scalar(out_ap, psum_ap, bias_ap, 0.0,
                                    op0=ALU.add, op1=ALU.max)

    with tile.TileContext(nc) as tc, ExitStack() as ctx:
        const = ctx.enter_context(tc.tile_pool(name="const", bufs=1))
        wpool = ctx.enter_context(tc.tile_pool(name="wts", bufs=4))
        apool = ctx.enter_context(tc.tile_pool(name="acts", bufs=1))
        inpool = ctx.enter_context(tc.tile_pool(name="inp", bufs=1))
        fpool = ctx.enter_context(tc.tile_pool(name="feat", bufs=2))
        ppool = ctx.enter_context(tc.tile_pool(name="ps", bufs=4, space="PSUM"))
        spool = ppool

        # constants: input filter, biases (all small, loaded once)
        fin_t = const.tile([128, C], BF16)
        nc.sync.dma_start(fin_t[:], p["fin"][:])
        bin_t = const.tile([128, 1], F32)
        nc.sync.dma_start(bin_t[:], p["bin"][:])
        bias_t = {}
        for lvl in range(1, NLVL + 1):
            bcols = LVL_NODES[lvl - 1] if lvl < NLVL else LVL_NODES[lvl - 1] // 2
            bias_t[lvl] = const.tile([128, bcols], F32, tag=f"bias{lvl}", name=f"bias{lvl}")
            nc.sync.dma_start(bias_t[lvl][:], p[f"b{lvl}"][:])

        for ph in range(PH):
            # ---------------- input staging ----------------
            a0s = inpool.tile([128, 2 * 64 * 64], BF16, tag="a0s", name=f"a0s{ph}")
            for g in range(4):
                nc.sync.dma_start(
                    a0s[g * 32 : g * 32 + 16, :],
                    p["a0"][g * 16 : (g + 1) * 16,
                            ph * 2 * 64 * 64 : (ph + 1) * 2 * 64 * 64],
                )
            a0v = a0s[:].rearrange("p (h x y) -> p h x y", h=2, x=64)

            # ---------------- input conv ----------------
            # X slab: [128=(y%2,c), (bl, x, y2)]  (bl=8, x=64, y2=32)
            X = apool.tile([128, BG * 64 * 32], BF16, tag="s0", name=f"x{ph}")
            Xv = X[:].rearrange("p (b h w) -> p b h w", b=BG, h=64)
            for bl in range(BG):
                g, half = bl % 4, bl // 4
                for xq in range(4):
                    pt = ppool.tile([128, 512], F32, tag="ps", padded_shape=[128, 512], name=f"pin{ph}_{bl}_{xq}")
                    for q in (0, 1):
                        rhs = a0v[g * 32 : g * 32 + 16, half,
                                  xq * 16 : (xq + 1) * 16, q::2]
                        nc.tensor.matmul(
                            pt[q * 64 : (q + 1) * 64, :],
                            fin_t[g * 32 : g * 32 + 16, :],
                            rhs,
                            start=True, stop=True,
                            tile_position=(g * 32, q * 64),
                        )
                    evict(Xv[:, bl, xq * 16 : (xq + 1) * 16, :], pt[:], bin_t[:, 0:1])

            # ---------------- levels 1..5 (q-scheme) ----------------
            cur = X          # slab with free = (node, bl, h, w2)
            cur_nodes = 1
            tags = ["s1", "s0", "s1", "s0", "s1"]
            for lvl in range(1, 6):
                n2 = LVL_NODES[lvl - 1]
                grid = int(np.sqrt(n2))
                Hin = LVL_HIN[lvl - 1]
                W2in = Hin // 2
                Ho, W2o = Hin // 2, W2in // 2
                ncols_out = BG * Ho * W2o
                nxt = apool.tile([128, n2 * ncols_out], BF16,
                                 tag=tags[lvl - 1], name=f"a{lvl}_{ph}")
                curv = cur[:].rearrange("p (n b h w) -> p n b h w",
                                        n=cur_nodes, b=BG, h=Hin)
                nxtv = nxt[:].rearrange("p (n b h w) -> p n b h w",
                                        n=n2, b=BG, h=Ho)
                # psum blocks of <=512 cols
                nblk = max(1, ncols_out // 512)
                bper = BG // nblk
                ncol = bper * Ho * W2o
                pgrid = int(np.sqrt(cur_nodes))
                for g0 in range(0, n2, WGRP):
                    gn = min(WGRP, n2 - g0)
                    wlt = wpool.tile([128, WGRP * 128], BF16, tag="wch",
                                     name=f"w{lvl}_{ph}_{g0}")
                    nc.sync.dma_start(
                        wlt[:, : gn * 128],
                        p[f"w{lvl}"][:, g0 * 128 : (g0 + gn) * 128],
                    )
                    for node in range(g0, g0 + gn):
                        u, v = node // grid, node % grid
                        ln = node - g0
                        pnode = ((u // 2) * pgrid + (v // 2)) if lvl <= KLVL else node
                        for blk in range(nblk):
                            bs = blk * bper
                            pt = ppool.tile([128, ncol], F32, tag="ps", padded_shape=[128, 512],
                                            name=f"p{lvl}_{ph}_{node}_{blk}")
                            for q in (0, 1):
                                for x in (0, 1):
                                    rhs = curv[:, pnode, bs : bs + bper, x::2, q::2]
                                    nc.tensor.matmul(
                                        pt[q * 64 : (q + 1) * 64, :],
                                        wlt[:, ln * 128 + x * 64 :
                                            ln * 128 + (x + 1) * 64],
                                        rhs,
                                        start=(x == 0), stop=(x == 1),
                                        tile_position=(0, q * 64),
                                    )
                            evict(
                                nxtv[:, node, bs : bs + bper, :, :],
                                pt[:],
                                bias_t[lvl][:, node : node + 1],
                            )
                cur = nxt
                cur_nodes = n2

            # ---------------- level 6 (node pairs, M=64) ----------------
            # cur: [128, (n=64, bl, h=2, w2=1)] ; feats F [64=c, (node, bl)]
            F = fpool.tile([64, NK * NK * BG], BF16, tag="feats", name=f"f{ph}")
            Fv = F[:].rearrange("c (n b) -> c n b", n=NK * NK)
            curv = cur[:].rearrange("p (n b h w) -> p n b h w", n=64, b=BG, h=2)
            for g0 in range(0, 64, WGRP):
                w6t = wpool.tile([128, WGRP * 128], BF16, tag="wch",
                                 name=f"w6_{ph}_{g0}")
                nc.sync.dma_start(
                    w6t[:], p["w6"][:, g0 * 128 : (g0 + WGRP) * 128]
                )
                for pr in range(g0 // 2, (g0 + WGRP) // 2):
                    nA, nB = 2 * pr, 2 * pr + 1
                    pt = spool.tile([128, BG], F32, tag="ps6", bufs=2, padded_shape=[128, 512],
                                    name=f"p6_{ph}_{pr}")
                    for half, node in ((0, nA), (1, nB)):
                        ln = node - g0
                        for x in (0, 1):
                            rhs = curv[:, node, :, x, 0]
                            nc.tensor.matmul(
                                pt[half * 64 : (half + 1) * 64, :],
                                w6t[:, ln * 128 + x * 64 :
                                    ln * 128 + (x + 1) * 64],
                                rhs,
                                start=(x == 0), stop=(x == 1),
                                tile_position=(0, half * 64),
                            )
                    bias_ap = bias_t[6][:, pr : pr + 1]
                    evict_ctr[0] += 1
                    if evict_ctr[0] % 2 == 0:
                        nc.scalar.activation(Fv[0:64, nA, :], pt[0:64, :], AF.Relu,
                                             bias=bias_ap[0:64, :])
                        nc.scalar.activation(Fv[0:64, nB, :], pt[64:128, :], AF.Relu,
                                             bias=bias_ap[64:128, :])
                    else:
                        nc.vector.tensor_scalar(Fv[0:64, nA, :], pt[0:64, :],
                                                bias_ap[0:64, :], 0.0,
                                                op0=ALU.add, op1=ALU.max)
                        nc.vector.tensor_scalar(Fv[0:64, nB, :], pt[64:128, :],
                                                bias_ap[64:128, :], 0.0,
                                                op0=ALU.add, op1=ALU.max)

            # ---------------- dense ----------------
            t2s = fpool.tile([128, NK * NK * BG], F32, tag="t2s", name=f"t2s{ph}")
            t2sv = t2s[:].rearrange("m (n b) -> m n b", n=NK * NK)
            for g0 in range(0, 64, WGRP):
                wdt = wpool.tile([64, WGRP * 128], BF16, tag="wdch",
                                 name=f"wd_{ph}_{g0}")
                nc.sync.dma_start(
                    wdt[:], p["wd"][:, g0 * 128 : (g0 + WGRP) * 128]
                )
                for node in range(g0, g0 + WGRP):
                    ln = node - g0
                    pt = spool.tile([128, BG], F32, tag="psd", bufs=2, padded_shape=[128, 512],
                                    name=f"pd_{ph}_{node}")
                    nc.tensor.matmul(
                        pt[:],
                        wdt[:, ln * 128 : (ln + 1) * 128],
                        Fv[:, node, :],
                        start=True, stop=True,
                    )
                    evict_ctr[0] += 1
                    if evict_ctr[0] % 2 == 0:
                        nc.scalar.copy(t2sv[:, node, :], pt[:])
                    else:
                        nc.vector.tensor_copy(t2sv[:, node, :], pt[:])
            nc.sync.dma_start(
                t2[:, ph * NK * NK * BG : (ph + 1) * NK * NK * BG], t2s[:]
            )
    nc.compile()
    return nc


# ----------------------------------------------------------------------------
# entry point
# ----------------------------------------------------------------------------

def kernel(**inputs):
    inputs = {k: np.asarray(v) for k, v in inputs.items()}
    wblobs = _prep_weights(inputs)
    nc = _build_kernel()
    in_maps = []
    for c in range(NCORES):
        m = dict(wblobs)
        m["a0"] = _prep_input(inputs["in_data"][c * BC : (c + 1) * BC])
        in_maps.append(m)
    res = run_bass_kernel_spmd(nc, in_maps, list(range(NCORES)))
    outs = [_decode_output(res.results[c]["t2"]) for c in range(NCORES)]
    return np.concatenate(outs, axis=0).astype(np.float32)


if __name__ == "__main__":
    import reference as ref

    inputs = {k: np.asarray(v) for k, v in ref.setup_inputs().items()}
    expected = np.asarray(ref.reference(**inputs))
    actual = kernel(**inputs)
    err = np.abs(actual - expected).max()
    rel = err / np.abs(expected).max()
    print("absmax:", err, "rel:", rel)


# revision 9
# speedup vs baseline: 368.6436x; 368.6436x over previous
"""Trainium2 Bass kernel for nn_ButterflyLayer2D (butterfly 2D CNN).

Strategy: pure data parallel over 8 NeuronCores (16 batch each), with the
per-core batch processed in 2 phases of 8 to fit SBUF.

All tensors are pre-arranged on the host (numpy) into DMA-friendly layouts:
  - activations live in SBUF as [128 = (w%2)*64 + c, (node, b, h, w//2)]
    so each 2x2-stride-2 per-node conv becomes 4 fp32r matmuls with K=128
    chunks: col-group q = output w-parity (tile_position (0, 64q)), x-chunks
    accumulate in PSUM. PSUM [128=(q,c_out), N] is evicted full-width by a
    single relu+bias op (alternating ScalarE/VectorE) directly into the next
    level's interleaved layout — zero data reshuffling anywhere on chip.
  - the input 4x4-patch conv uses the same trick with K=16 row-groups spread
    over 4 partition groups (one per b%4) for PE concurrency.
  - the final per-node dense is a [64,128] x [64,b] matmul; outputs are
    written as [128=(r,ou,ov), (ph,node,b)] and decoded on the host.
Weights are streamed from HBM in 8-node chunks through a recycled tile tag.
"""

import numpy as np
from contextlib import ExitStack

import concourse.bass as bass
import concourse.tile as tile
from concourse import bacc, mybir
from concourse.bass_utils import run_bass_kernel_spmd

F32 = mybir.dt.float32
F32R = mybir.dt.float32r
BF16 = mybir.dt.bfloat16
AF = mybir.ActivationFunctionType
ALU = mybir.AluOpType

B, IN, NLVL, KLVL, C = 128, 256, 6, 3, 64
NK, OU, OV = 8, 8, 8
NCORES = 8
BC = B // NCORES          # 16 per-core batch
PH = 2                    # phases per core
BG = BC // PH             # 8 batch per phase
LVL_NODES = [4, 16, 64, 64, 64, 64]          # nodes per level
LVL_HIN = [64, 32, 16, 8, 4, 2]              # spatial H into each level
WGRP = 8                  # weight streaming chunk (nodes)


# ----------------------------------------------------------------------------
# host-side pre-arrangement
# ----------------------------------------------------------------------------

def _prep_weights(inputs):
    """Weights/biases blobs shared by all cores."""
    out = {}
    # input filter: lhsT [16=(p,q), 64], replicated at partition bases 0/32/64/96
    import ml_dtypes
    fin = inputs["in_filter"][:, :, 0, :].reshape(16, C).astype(np.float32)
    finr = np.zeros((128, C), np.float32)
    for g in range(4):
        finr[g * 32 : g * 32 + 16] = fin
    out["fin"] = finr.astype(ml_dtypes.bfloat16)
    out["bin"] = np.concatenate([inputs["in_bias"], inputs["in_bias"]]).reshape(
        128, 1
    ).astype(np.float32)

    for lvl in range(1, NLVL + 1):
        f = inputs[f"f{lvl}"].astype(np.float32)  # [n,n,2,2,C,C] (x,y,ci,co)
        n = f.shape[0]
        assert n == 2 ** min(lvl, KLVL)
        # lhsT per node: [(y*64+ci), (x*64+co)]
        w = f.transpose(0, 1, 3, 4, 2, 5).reshape(n * n, 2 * C, 2 * C)
        # blob [128, nodes*128], free = (node, x*64+co)
        out[f"w{lvl}"] = np.ascontiguousarray(w.transpose(1, 0, 2)).reshape(
            128, n * n * 128
        ).astype(ml_dtypes.bfloat16)
        b = inputs[f"b{lvl}"].astype(np.float32).reshape(n * n, C)
        if lvl < NLVL:
            # [128, nodes]: rows (q,c) with bias duplicated across q
            bb = np.concatenate([b, b], axis=1)  # [nodes, 128]
            out[f"b{lvl}"] = np.ascontiguousarray(bb.T)
        else:
            # lvl6 node-pair scheme: psum rows = (cA, cB) for pair (2k, 2k+1)
            bb = b.reshape(n * n // 2, 2 * C)  # [pairs, (cA,cB)]
            out[f"b{lvl}"] = np.ascontiguousarray(bb.T)  # [128, 32]
    # dense: lhsT per node [64=c, 128=(r, ou*8+ov)]
    wd = inputs["Wd"].astype(np.float32).reshape(NK * NK, 2, C, OU * OV)
    wd = wd.transpose(2, 0, 1, 3).reshape(C, NK * NK * 2 * OU * OV)
    out["wd"] = np.ascontiguousarray(wd).astype(ml_dtypes.bfloat16)
    return out


def _prep_input(in_data_core):
    """Per-core input blob: [64 = (b%4)*16 + (i%4)*4 + (j%4),
    (ph, b//4%2, x=i//4, y4=j//4)] packed (no zero rows)."""
    ind = in_data_core[:, :, :, 0]  # [16, 256, 256]
    a = ind.reshape(PH, 2, 4, 64, 4, 64, 4)  # [ph, half, g, x, p, y4, q]
    a = a.transpose(2, 4, 6, 0, 1, 3, 5)     # [g, p, q, ph, half, x, y4]
    import ml_dtypes
    return np.ascontiguousarray(a).reshape(64, PH * 2 * 64 * 64).astype(ml_dtypes.bfloat16)


def _decode_output(t2_core):
    """t2 [128=(r,ou,ov), (ph, node, bl)] -> [16, 64, 64, 2]."""
    t = t2_core.reshape(2, OU, OV, PH, NK, NK, BG)  # r,ou,ov,ph,u,v,bl
    t = t.transpose(3, 6, 4, 1, 5, 2, 0)            # ph,bl,u,ou,v,ov,r
    return np.ascontiguousarray(t).reshape(BC, NK * OU, NK * OV, 2)


# ----------------------------------------------------------------------------
# device kernel
# ----------------------------------------------------------------------------

def _build_kernel(reps=1, xouter=True):
    nc = bacc.Bacc(None, target_bir_lowering=False)
    p = {}
    p["a0"] = nc.declare_dram_parameter("a0", [64, PH * 2 * 64 * 64], BF16, isOutput=False)
    p["fin"] = nc.declare_dram_parameter("fin", [128, C], BF16, isOutput=False)
    p["bin"] = nc.declare_dram_parameter("bin", [128, 1], F32, isOutput=False)
    for lvl in range(1, NLVL + 1):
        n2 = LVL_NODES[lvl - 1]
        p[f"w{lvl}"] = nc.declare_dram_parameter(f"w{lvl}", [128, n2 * 128], BF16, isOutput=False)
        bcols = n2 if lvl < NLVL else n2 // 2
        p[f"b{lvl}"] = nc.declare_dram_parameter(f"b{lvl}", [128, bcols], F32, isOutput=False)
    p["wd"] = nc.declare_dram_parameter("wd", [64, NK * NK * 128], BF16, isOutput=False)
    t2 = nc.declare_dram_parameter("t2", [128, PH * NK * NK * BG], F32, isOutput=True)

    evict_ctr = [0]

    def evict(out_ap, psum_ap, bias_ap):
        """relu(psum + bias) -> sbuf, alternating engines to split the load."""
        evict_ctr[0] += 1
        if evict_ctr[0] % 2 == 0:
            nc.scalar.activation(out_ap, psum_ap, AF.Relu, bias=bias_ap)
        else:
            nc.vector.tensor_scalar(out_ap, psum_ap, bias_ap, 0.0,
                                    op0=ALU.add, op1=ALU.max)

    with tile.TileContext(nc) as tc, ExitStack() as ctx:
        const = ctx.enter_context(tc.tile_pool(name="const", bufs=1))
        wpool = ctx.enter_context(tc.tile_pool(name="wts", bufs=4))
        apool = ctx.enter_context(tc.tile_pool(name="acts", bufs=1))
        inpool = ctx.enter_context(tc.tile_pool(name="inp", bufs=1))
        fpool = ctx.enter_context(tc.tile_pool(name="feat", bufs=2))
        ppool = ctx.enter_context(tc.tile_pool(name="ps", bufs=4, space="PSUM"))
        spool = ppool

        # constants: input filter, biases (all small, loaded once)
        fin_t = const.tile([128, C], BF16)
        nc.sync.dma_start(fin_t[:], p["fin"][:])
        bin_t = const.tile([128, 1], F32)
        nc.sync.dma_start(bin_t[:], p["bin"][:])
        bias_t = {}
        for lvl in range(1, NLVL + 1):
            bcols = LVL_NODES[lvl - 1] if lvl < NLVL else LVL_NODES[lvl - 1] // 2
            bias_t[lvl] = const.tile([128, bcols], F32, tag=f"bias{lvl}", name=f"bias{lvl}")
            nc.sync.dma_start(bias_t[lvl][:], p[f"b{lvl}"][:])

        for phx in range(reps * PH):
            ph = phx % PH
            # ---------------- input staging ----------------
            a0s = inpool.tile([128, 2 * 64 * 64], BF16, tag="a0s", name=f"a0s{phx}")
            for g in range(4):
                nc.sync.dma_start(
                    a0s[g * 32 : g * 32 + 16, :],
                    p["a0"][g * 16 : (g + 1) * 16,
                            ph * 2 * 64 * 64 : (ph + 1) * 2 * 64 * 64],
                )
            a0v = a0s[:].rearrange("p (h x y) -> p h x y", h=2, x=64)

            # ---------------- input conv ----------------
            # X slab: [128=(y%2,c), (bl, x, y2)]  (bl=8, x=64, y2=32)
            X = apool.tile([128, BG * 64 * 32], BF16, tag="s0", name=f"x{phx}")
            Xv = X[:].rearrange("p (b h w) -> p b h w", b=BG, h=64)
            for bl in range(BG):
                g, half = bl % 4, bl // 4
                for xq in range(4):
                    pt = ppool.tile([128, 512], F32, tag="ps", padded_shape=[128, 512], name=f"pin{phx}_{bl}_{xq}")
                    for q in (0, 1):
                        rhs = a0v[g * 32 : g * 32 + 16, half,
                                  xq * 16 : (xq + 1) * 16, q::2]
                        nc.tensor.matmul(
                            pt[q * 64 : (q + 1) * 64, :],
                            fin_t[g * 32 : g * 32 + 16, :],
                            rhs,
                            start=True, stop=True,
                            tile_position=(g * 32, q * 64),
                        )
                    evict(Xv[:, bl, xq * 16 : (xq + 1) * 16, :], pt[:], bin_t[:, 0:1])

            # ---------------- levels 1..5 (q-scheme) ----------------
            cur = X          # slab with free = (node, bl, h, w2)
            cur_nodes = 1
            tags = ["s1", "s0", "s1", "s0", "s1"]
            for lvl in range(1, 6):
                n2 = LVL_NODES[lvl - 1]
                grid = int(np.sqrt(n2))
                Hin = LVL_HIN[lvl - 1]
                W2in = Hin // 2
                Ho, W2o = Hin // 2, W2in // 2
                ncols_out = BG * Ho * W2o
                nxt = apool.tile([128, n2 * ncols_out], BF16,
                                 tag=tags[lvl - 1], name=f"a{lvl}_{phx}")
                curv = cur[:].rearrange("p (n b h w) -> p n b h w",
                                        n=cur_nodes, b=BG, h=Hin)
                nxtv = nxt[:].rearrange("p (n b h w) -> p n b h w",
                                        n=n2, b=BG, h=Ho)
                # psum blocks of <=512 cols
                nblk = max(1, ncols_out // 512)
                bper = BG // nblk
                ncol = bper * Ho * W2o
                pgrid = int(np.sqrt(cur_nodes))
                for g0 in range(0, n2, WGRP):
                    gn = min(WGRP, n2 - g0)
                    wlt = wpool.tile([128, WGRP * 128], BF16, tag="wch",
                                     name=f"w{lvl}_{phx}_{g0}")
                    nc.sync.dma_start(
                        wlt[:, : gn * 128],
                        p[f"w{lvl}"][:, g0 * 128 : (g0 + gn) * 128],
                    )
                    for node in range(g0, g0 + gn):
                        u, v = node // grid, node % grid
                        ln = node - g0
                        pnode = ((u // 2) * pgrid + (v // 2)) if lvl <= KLVL else node
                        for blk in range(nblk):
                            bs = blk * bper
                            pt = ppool.tile([128, ncol], F32, tag="ps", padded_shape=[128, 512],
                                            name=f"p{lvl}_{phx}_{node}_{blk}")
                            qx = [(x, q) for x in (0, 1) for q in (0, 1)] \
                                if xouter else \
                                [(x, q) for q in (0, 1) for x in (0, 1)]
                            for x, q in qx:
                                rhs = curv[:, pnode, bs : bs + bper, x::2, q::2]
                                nc.tensor.matmul(
                                    pt[q * 64 : (q + 1) * 64, :],
                                    wlt[:, ln * 128 + x * 64 :
                                        ln * 128 + (x + 1) * 64],
                                    rhs,
                                    start=(x == 0), stop=(x == 1),
                                    skip_group_check=xouter,
                                    tile_position=(0, q * 64),
                                )
                            evict(
                                nxtv[:, node, bs : bs + bper, :, :],
                                pt[:],
                                bias_t[lvl][:, node : node + 1],
                            )
                cur = nxt
                cur_nodes = n2

            # ---------------- level 6 (node pairs, M=64) ----------------
            # cur: [128, (n=64, bl, h=2, w2=1)] ; feats F [64=c, (node, bl)]
            F = fpool.tile([64, NK * NK * BG], BF16, tag="feats", name=f"f{phx}")
            Fv = F[:].rearrange("c (n b) -> c n b", n=NK * NK)
            curv = cur[:].rearrange("p (n b h w) -> p n b h w", n=64, b=BG, h=2)
            for g0 in range(0, 64, WGRP):
                w6t = wpool.tile([128, WGRP * 128], BF16, tag="wch",
                                 name=f"w6_{phx}_{g0}")
                nc.sync.dma_start(
                    w6t[:], p["w6"][:, g0 * 128 : (g0 + WGRP) * 128]
                )
                for pr in range(g0 // 2, (g0 + WGRP) // 2):
                    nA, nB = 2 * pr, 2 * pr + 1
                    pt = spool.tile([128, BG], F32, tag="ps6", bufs=2, padded_shape=[128, 512],
                                    name=f"p6_{phx}_{pr}")
                    hx = [(h_, x_) for x_ in (0, 1) for h_ in (0, 1)] \
                        if xouter else \
                        [(h_, x_) for h_ in (0, 1) for x_ in (0, 1)]
                    for half, x in hx:
                        node = nA if half == 0 else nB
                        ln = node - g0
                        rhs = curv[:, node, :, x, 0]
                        nc.tensor.matmul(
                            pt[half * 64 : (half + 1) * 64, :],
                            w6t[:, ln * 128 + x * 64 :
                                ln * 128 + (x + 1) * 64],
                            rhs,
                            start=(x == 0), stop=(x == 1),
                            skip_group_check=xouter,
                            tile_position=(0, half * 64),
                        )
                    bias_ap = bias_t[6][:, pr : pr + 1]
                    evict_ctr[0] += 1
                    if evict_ctr[0] % 2 == 0:
                        nc.scalar.activation(Fv[0:64, nA, :], pt[0:64, :], AF.Relu,
                                             bias=bias_ap[0:64, :])
                        nc.scalar.activation(Fv[0:64, nB, :], pt[64:128, :], AF.Relu,
                                             bias=bias_ap[64:128, :])
                    else:
                        nc.vector.tensor_scalar(Fv[0:64, nA, :], pt[0:64, :],
                                                bias_ap[0:64, :], 0.0,
                                                op0=ALU.add, op1=ALU.max)
                        nc.vector.tensor_scalar(Fv[0:64, nB, :], pt[64:128, :],
                                                bias_ap[64:128, :], 0.0,
                                                op0=ALU.add, op1=ALU.max)

            # ---------------- dense ----------------
            t2s = fpool.tile([128, NK * NK * BG], F32, tag="t2s", name=f"t2s{phx}")
            t2sv = t2s[:].rearrange("m (n b) -> m n b", n=NK * NK)
            for g0 in range(0, 64, WGRP):
                wdt = wpool.tile([64, WGRP * 128], BF16, tag="wdch",
                                 name=f"wd_{phx}_{g0}")
                nc.sync.dma_start(
                    wdt[:], p["wd"][:, g0 * 128 : (g0 + WGRP) * 128]
                )
                for node in range(g0, g0 + WGRP):
                    ln = node - g0
                    pt = spool.tile([128, BG], F32, tag="psd", bufs=2, padded_shape=[128, 512],
                                    name=f"pd_{phx}_{node}")
                    nc.tensor.matmul(
                        pt[:],
                        wdt[:, ln * 128 : (ln + 1) * 128],
                        Fv[:, node, :],
                        start=True, stop=True,
                    )
                    evict_ctr[0] += 1
                    if evict_ctr[0] % 2 == 0:
                        nc.scalar.copy(t2sv[:, node, :], pt[:])
                    else:
                        nc.vector.tensor_copy(t2sv[:, node, :], pt[:])
            nc.sync.dma_start(
                t2[:, ph * NK * NK * BG : (ph + 1) * NK * NK * BG], t2s[:]
            )
    nc.compile()
    return nc


# ----------------------------------------------------------------------------
# entry point
# ----------------------------------------------------------------------------

def kernel(**inputs):
    inputs = {k: np.asarray(v) for k, v in inputs.items()}
    wblobs = _prep_weights(inputs)
    nc = _build_kernel()
    in_maps = []
    for c in range(NCORES):
        m = dict(wblobs)
        m["a0"] = _prep_input(inputs["in_data"][c * BC : (c + 1) * BC])
        in_maps.append(m)
    res = run_bass_kernel_spmd(nc, in_maps, list(range(NCORES)))
    outs = [_decode_output(res.results[c]["t2"]) for c in range(NCORES)]
    return np.concatenate(outs, axis=0).astype(np.float32)


if __name__ == "__main__":
    import reference as ref

    inputs = {k: np.asarray(v) for k, v in ref.setup_inputs().items()}
    expected = np.asarray(ref.reference(**inputs))
    actual = kernel(**inputs)
    err = np.abs(actual - expected).max()
    rel = err / np.abs(expected).max()
    print("absmax:", err, "rel:", rel)


# revision 10
# speedup vs baseline: 392.5727x; 1.0649x over previous
"""Trainium2 Bass kernel for nn_ButterflyLayer2D (butterfly 2D CNN).

Strategy: pure data parallel over 8 NeuronCores (16 batch each), with the
per-core batch processed in 2 phases of 8 to fit SBUF.

All tensors are pre-arranged on the host (numpy) into DMA-friendly layouts:
  - activations live in SBUF as [128 = (w%2)*64 + c, (node, b, h, w//2)]
    so each 2x2-stride-2 per-node conv becomes 4 fp32r matmuls with K=128
    chunks: col-group q = output w-parity (tile_position (0, 64q)), x-chunks
    accumulate in PSUM. PSUM [128=(q,c_out), N] is evicted full-width by a
    single relu+bias op (alternating ScalarE/VectorE) directly into the next
    level's interleaved layout — zero data reshuffling anywhere on chip.
  - the input 4x4-patch conv uses the same trick with K=16 row-groups spread
    over 4 partition groups (one per b%4) for PE concurrency.
  - the final per-node dense is a [64,128] x [64,b] matmul; outputs are
    written as [128=(r,ou,ov), (ph,node,b)] and decoded on the host.
Weights are streamed from HBM in 8-node chunks through a recycled tile tag.
"""

import numpy as np
from contextlib import ExitStack

import concourse.bass as bass
import concourse.tile as tile
from concourse import bacc, mybir
from concourse.bass_utils import run_bass_kernel_spmd

F32 = mybir.dt.float32
F32R = mybir.dt.float32r
BF16 = mybir.dt.bfloat16
AF = mybir.ActivationFunctionType
ALU = mybir.AluOpType

B, IN, NLVL, KLVL, C = 128, 256, 6, 3, 64
NK, OU, OV = 8, 8, 8
NCORES = 8
BC = B // NCORES          # 16 per-core batch
PH = 1                    # phases per core
BG = BC // PH             # batch per phase
HALF = BG // 4            # input-conv b-subgroups per partition group
LVL_NODES = [4, 16, 64, 64, 64, 64]          # nodes per level
LVL_HIN = [64, 32, 16, 8, 4, 2]              # spatial H into each level
WGRP = 8                  # weight streaming chunk (nodes)


# ----------------------------------------------------------------------------
# host-side pre-arrangement
# ----------------------------------------------------------------------------

def _prep_weights(inputs):
    """Weights/biases blobs shared by all cores."""
    out = {}
    # input filter: lhsT [16=(p,q), 64], replicated at partition bases 0/32/64/96
    import ml_dtypes
    fin = inputs["in_filter"][:, :, 0, :].reshape(16, C).astype(np.float32)
    finr = np.zeros((128, C), np.float32)
    for g in range(4):
        finr[g * 32 : g * 32 + 16] = fin
    out["fin"] = finr.astype(ml_dtypes.bfloat16)
    out["bin"] = np.concatenate([inputs["in_bias"], inputs["in_bias"]]).reshape(
        128, 1
    ).astype(np.float32)

    for lvl in range(1, NLVL + 1):
        f = inputs[f"f{lvl}"].astype(np.float32)  # [n,n,2,2,C,C] (x,y,ci,co)
        n = f.shape[0]
        assert n == 2 ** min(lvl, KLVL)
        # lhsT per node: [(y*64+ci), (x*64+co)]
        w = f.transpose(0, 1, 3, 4, 2, 5).reshape(n * n, 2 * C, 2 * C)
        # blob [128, nodes*128], free = (node, x*64+co)
        out[f"w{lvl}"] = np.ascontiguousarray(w.transpose(1, 0, 2)).reshape(
            128, n * n * 128
        ).astype(ml_dtypes.bfloat16)
        b = inputs[f"b{lvl}"].astype(np.float32).reshape(n * n, C)
        if lvl < NLVL:
            # [128, nodes]: rows (q,c) with bias duplicated across q
            bb = np.concatenate([b, b], axis=1)  # [nodes, 128]
            out[f"b{lvl}"] = np.ascontiguousarray(bb.T)
        else:
            # lvl6 node-pair scheme: psum rows = (cA, cB) for pair (2k, 2k+1)
            bb = b.reshape(n * n // 2, 2 * C)  # [pairs, (cA,cB)]
            out[f"b{lvl}"] = np.ascontiguousarray(bb.T)  # [128, 32]
    # dense: lhsT per node [64=c, 128=(r, ou*8+ov)]
    wd = inputs["Wd"].astype(np.float32).reshape(NK * NK, 2, C, OU * OV)
    wd = wd.transpose(2, 0, 1, 3).reshape(C, NK * NK * 2 * OU * OV)
    out["wd"] = np.ascontiguousarray(wd).astype(ml_dtypes.bfloat16)
    return out


def _prep_input(in_data_core):
    """Per-core input blob: [64 = (b%4)*16 + (i%4)*4 + (j%4),
    (ph, b//4%2, x=i//4, y4=j//4)] packed (no zero rows)."""
    ind = in_data_core[:, :, :, 0]  # [16, 256, 256]
    a = ind.reshape(PH, HALF, 4, 64, 4, 64, 4)  # [ph, half, g, x, p, y4, q]
    a = a.transpose(2, 4, 6, 0, 1, 3, 5)        # [g, p, q, ph, half, x, y4]
    import ml_dtypes
    return np.ascontiguousarray(a).reshape(64, PH * HALF * 64 * 64).astype(ml_dtypes.bfloat16)


def _decode_output(t2_core):
    """t2 [128=(r,ou,ov), (ph, node, bl)] -> [16, 64, 64, 2]."""
    t = t2_core.reshape(2, OU, OV, PH, NK, NK, BG)  # r,ou,ov,ph,u,v,bl
    t = t.transpose(3, 6, 4, 1, 5, 2, 0)            # ph,bl,u,ou,v,ov,r
    return np.ascontiguousarray(t).reshape(BC, NK * OU, NK * OV, 2)


# ----------------------------------------------------------------------------
# device kernel
# ----------------------------------------------------------------------------

def _build_kernel(reps=1, xouter=True):
    nc = bacc.Bacc(None, target_bir_lowering=False)
    p = {}
    p["a0"] = nc.declare_dram_parameter("a0", [64, PH * HALF * 64 * 64], BF16, isOutput=False)
    p["fin"] = nc.declare_dram_parameter("fin", [128, C], BF16, isOutput=False)
    p["bin"] = nc.declare_dram_parameter("bin", [128, 1], F32, isOutput=False)
    for lvl in range(1, NLVL + 1):
        n2 = LVL_NODES[lvl - 1]
        p[f"w{lvl}"] = nc.declare_dram_parameter(f"w{lvl}", [128, n2 * 128], BF16, isOutput=False)
        bcols = n2 if lvl < NLVL else n2 // 2
        p[f"b{lvl}"] = nc.declare_dram_parameter(f"b{lvl}", [128, bcols], F32, isOutput=False)
    p["wd"] = nc.declare_dram_parameter("wd", [64, NK * NK * 128], BF16, isOutput=False)
    t2 = nc.declare_dram_parameter("t2", [128, PH * NK * NK * BG], F32, isOutput=True)

    evict_ctr = [0]

    def evict(out_ap, psum_ap, bias_ap):
        """relu(psum + bias) -> sbuf, alternating engines to split the load."""
        evict_ctr[0] += 1
        if evict_ctr[0] % 2 == 0:
            nc.scalar.activation(out_ap, psum_ap, AF.Relu, bias=bias_ap)
        else:
            nc.vector.tensor_scalar(out_ap, psum_ap, bias_ap, 0.0,
                                    op0=ALU.add, op1=ALU.max)

    with tile.TileContext(nc) as tc, ExitStack() as ctx:
        const = ctx.enter_context(tc.tile_pool(name="const", bufs=1))
        wpool = ctx.enter_context(tc.tile_pool(name="wts", bufs=4))
        apool = ctx.enter_context(tc.tile_pool(name="acts", bufs=1))
        inpool = ctx.enter_context(tc.tile_pool(name="inp", bufs=1))
        fpool = ctx.enter_context(tc.tile_pool(name="feat", bufs=2))
        ppool = ctx.enter_context(tc.tile_pool(name="ps", bufs=4, space="PSUM"))
        spool = ppool

        # constants: input filter, biases (all small, loaded once)
        fin_t = const.tile([128, C], BF16)
        nc.sync.dma_start(fin_t[:], p["fin"][:])
        bin_t = const.tile([128, 1], F32)
        nc.sync.dma_start(bin_t[:], p["bin"][:])
        bias_t = {}
        for lvl in range(1, NLVL + 1):
            bcols = LVL_NODES[lvl - 1] if lvl < NLVL else LVL_NODES[lvl - 1] // 2
            bias_t[lvl] = const.tile([128, bcols], F32, tag=f"bias{lvl}", name=f"bias{lvl}")
            nc.sync.dma_start(bias_t[lvl][:], p[f"b{lvl}"][:])

        for phx in range(reps * PH):
            ph = phx % PH
            # ---------------- input staging ----------------
            a0s = inpool.tile([128, HALF * 64 * 64], BF16, tag="a0s", name=f"a0s{phx}")
            for g in range(4):
                nc.sync.dma_start(
                    a0s[g * 32 : g * 32 + 16, :],
                    p["a0"][g * 16 : (g + 1) * 16,
                            ph * HALF * 64 * 64 : (ph + 1) * HALF * 64 * 64],
                )
            a0v = a0s[:].rearrange("p (h x y) -> p h x y", h=HALF, x=64)

            # ---------------- input conv ----------------
            # X slab: [128=(y%2,c), (bl, x, y2)]  (bl=8, x=64, y2=32)
            X = apool.tile([128, BG * 64 * 32], BF16, tag="s0", name=f"x{phx}")
            Xv = X[:].rearrange("p (b h w) -> p b h w", b=BG, h=64)
            for bl in range(BG):
                g, half = bl % 4, bl // 4
                for xq in range(4):
                    pt = ppool.tile([128, 512], F32, tag="ps", padded_shape=[128, 512], name=f"pin{phx}_{bl}_{xq}")
                    for q in (0, 1):
                        rhs = a0v[g * 32 : g * 32 + 16, half,
                                  xq * 16 : (xq + 1) * 16, q::2]
                        nc.tensor.matmul(
                            pt[q * 64 : (q + 1) * 64, :],
                            fin_t[g * 32 : g * 32 + 16, :],
                            rhs,
                            start=True, stop=True,
                            tile_position=(g * 32, q * 64),
                        )
                    evict(Xv[:, bl, xq * 16 : (xq + 1) * 16, :], pt[:], bin_t[:, 0:1])

            # ---------------- levels 1..5 (q-scheme) ----------------
            cur = X          # slab with free = (node, bl, h, w2)
            cur_nodes = 1
            tags = ["s1", "s0", "s1", "s0", "s1"]
            for lvl in range(1, 6):
                n2 = LVL_NODES[lvl - 1]
                grid = int(np.sqrt(n2))
                Hin = LVL_HIN[lvl - 1]
                W2in = Hin // 2
                Ho, W2o = Hin // 2, W2in // 2
                ncols_out = BG * Ho * W2o
                nxt = apool.tile([128, n2 * ncols_out], BF16,
                                 tag=tags[lvl - 1], name=f"a{lvl}_{phx}")
                curv = cur[:].rearrange("p (n b h w) -> p n b h w",
                                        n=cur_nodes, b=BG, h=Hin)
                nxtv = nxt[:].rearrange("p (n b h w) -> p n b h w",
                                        n=n2, b=BG, h=Ho)
                # psum blocks of <=512 cols
                nblk = max(1, ncols_out // 512)
                bper = BG // nblk
                ncol = bper * Ho * W2o
                pgrid = int(np.sqrt(cur_nodes))
                for g0 in range(0, n2, WGRP):
                    gn = min(WGRP, n2 - g0)
                    wlt = wpool.tile([128, WGRP * 128], BF16, tag="wch",
                                     name=f"w{lvl}_{phx}_{g0}")
                    nc.sync.dma_start(
                        wlt[:, : gn * 128],
                        p[f"w{lvl}"][:, g0 * 128 : (g0 + gn) * 128],
                    )
                    for node in range(g0, g0 + gn):
                        u, v = node // grid, node % grid
                        ln = node - g0
                        pnode = ((u // 2) * pgrid + (v // 2)) if lvl <= KLVL else node
                        for blk in range(nblk):
                            bs = blk * bper
                            pt = ppool.tile([128, ncol], F32, tag="ps", padded_shape=[128, 512],
                                            name=f"p{lvl}_{phx}_{node}_{blk}")
                            qx = [(x, q) for x in (0, 1) for q in (0, 1)] \
                                if xouter else \
                                [(x, q) for q in (0, 1) for x in (0, 1)]
                            for x, q in qx:
                                rhs = curv[:, pnode, bs : bs + bper, x::2, q::2]
                                nc.tensor.matmul(
                                    pt[q * 64 : (q + 1) * 64, :],
                                    wlt[:, ln * 128 + x * 64 :
                                        ln * 128 + (x + 1) * 64],
                                    rhs,
                                    start=(x == 0), stop=(x == 1),
                                    skip_group_check=xouter,
                                    tile_position=(0, q * 64),
                                )
                            evict(
                                nxtv[:, node, bs : bs + bper, :, :],
                                pt[:],
                                bias_t[lvl][:, node : node + 1],
                            )
                cur = nxt
                cur_nodes = n2

            # ---------------- level 6 (node pairs, M=64) ----------------
            # cur: [128, (n=64, bl, h=2, w2=1)] ; feats F [64=c, (node, bl)]
            F = fpool.tile([64, NK * NK * BG], BF16, tag="feats", name=f"f{phx}")
            Fv = F[:].rearrange("c (n b) -> c n b", n=NK * NK)
            curv = cur[:].rearrange("p (n b h w) -> p n b h w", n=64, b=BG, h=2)
            for g0 in range(0, 64, WGRP):
                w6t = wpool.tile([128, WGRP * 128], BF16, tag="wch",
                                 name=f"w6_{phx}_{g0}")
                nc.sync.dma_start(
                    w6t[:], p["w6"][:, g0 * 128 : (g0 + WGRP) * 128]
                )
                for pr in range(g0 // 2, (g0 + WGRP) // 2):
                    nA, nB = 2 * pr, 2 * pr + 1
                    pt = spool.tile([128, BG], F32, tag="ps6", bufs=2, padded_shape=[128, 512],
                                    name=f"p6_{phx}_{pr}")
                    hx = [(h_, x_) for x_ in (0, 1) for h_ in (0, 1)] \
                        if xouter else \
                        [(h_, x_) for h_ in (0, 1) for x_ in (0, 1)]
                    for half, x in hx:
                        node = nA if half == 0 else nB
                        ln = node - g0
                        rhs = curv[:, node, :, x, 0]
                        nc.tensor.matmul(
                            pt[half * 64 : (half + 1) * 64, :],
                            w6t[:, ln * 128 + x * 64 :
                                ln * 128 + (x + 1) * 64],
                            rhs,
                            start=(x == 0), stop=(x == 1),
                            skip_group_check=xouter,
                            tile_position=(0, half * 64),
                        )
                    bias_ap = bias_t[6][:, pr : pr + 1]
                    evict_ctr[0] += 1
                    if evict_ctr[0] % 2 == 0:
                        nc.scalar.activation(Fv[0:64, nA, :], pt[0:64, :], AF.Relu,
                                             bias=bias_ap[0:64, :])
                        nc.scalar.activation(Fv[0:64, nB, :], pt[64:128, :], AF.Relu,
                                             bias=bias_ap[64:128, :])
                    else:
                        nc.vector.tensor_scalar(Fv[0:64, nA, :], pt[0:64, :],
                                                bias_ap[0:64, :], 0.0,
                                                op0=ALU.add, op1=ALU.max)
                        nc.vector.tensor_scalar(Fv[0:64, nB, :], pt[64:128, :],
                                                bias_ap[64:128, :], 0.0,
                                                op0=ALU.add, op1=ALU.max)

            # ---------------- dense ----------------
            t2s = fpool.tile([128, NK * NK * BG], F32, tag="t2s", name=f"t2s{phx}")
            t2sv = t2s[:].rearrange("m (n b) -> m n b", n=NK * NK)
            for g0 in range(0, 64, WGRP):
                wdt = wpool.tile([64, WGRP * 128], BF16, tag="wdch",
                                 name=f"wd_{phx}_{g0}")
                nc.sync.dma_start(
                    wdt[:], p["wd"][:, g0 * 128 : (g0 + WGRP) * 128]
                )
                for node in range(g0, g0 + WGRP):
                    ln = node - g0
                    pt = spool.tile([128, BG], F32, tag="psd", bufs=2, padded_shape=[128, 512],
                                    name=f"pd_{phx}_{node}")
                    nc.tensor.matmul(
                        pt[:],
                        wdt[:, ln * 128 : (ln + 1) * 128],
                        Fv[:, node, :],
                        start=True, stop=True,
                    )
                    evict_ctr[0] += 1
                    if evict_ctr[0] % 2 == 0:
                        nc.scalar.copy(t2sv[:, node, :], pt[:])
                    else:
                        nc.vector.tensor_copy(t2sv[:, node, :], pt[:])
            nc.sync.dma_start(
                t2[:, ph * NK * NK * BG : (ph + 1) * NK * NK * BG], t2s[:]
            )
    nc.compile()
    return nc


# ----------------------------------------------------------------------------
# entry point
# ----------------------------------------------------------------------------

def kernel(**inputs):
    inputs = {k: np.asarray(v) for k, v in inputs.items()}
    wblobs = _prep_weights(inputs)
    nc = _build_kernel()
    in_maps = []
    for c in range(NCORES):
        m = dict(wblobs)
        m["a0"] = _prep_input(inputs["in_data"][c * BC : (c + 1) * BC])
        in_maps.append(m)
    res = run_bass_kernel_spmd(nc, in_maps, list(range(NCORES)))
    outs = [_decode_output(res.results[c]["t2"]) for c in range(NCORES)]
    return np.concatenate(outs, axis=0).astype(np.float32)


if __name__ == "__main__":
    import reference as ref

    inputs = {k: np.asarray(v) for k, v in ref.setup_inputs().items()}
    expected = np.asarray(ref.reference(**inputs))
    actual = kernel(**inputs)
    err = np.abs(actual - expected).max()
    rel = err / np.abs(expected).max()
    print("absmax:", err, "rel:", rel)
